# revision 25
# baseline (speedup 1.0000x reference)
"""CON_GATLayer Trainium2 kernel: 8-core row-sharded GAT with dual-branch
score gathering via a two-level compact local_scatter scheme.

Self-contained: host preprocessing (index scatter-schedules, weight
augmentation) + Bass/Tile kernel builder + SPMD runner.

Gather scheme (per 128-row tile, head, branch): the softmax weights need
G[i,j] = exp(att_other[i, idx[i,j]]) at edge positions.  A first
"build" local_scatter scans the N-wide exp stream once and compacts
every value that has >=1 target into a <=2046-wide region C laid out as
[A-only | SHARED | B-only] (A/B = destination column halves, the dst of
one scatter is limited to 2047 elements).  Values are classified per
(row, source) by their target counts (kA, kB) into Q (A-only), P
(kA=1,kB>=1), S (kA>=2,kB=1), T (kA>=2,kB>=2), R (B-only); duplicate
deliveries come from prefix copies (DVE) of the count-sorted regions.
The two half-scatters then scan only [A-only+SHARED] and [SHARED+B-only]
(~1000 columns each) instead of the naive (N + tail) per half.
"""
import math
import numpy as np

import concourse.bass as bass
import concourse.tile as tile
from concourse import bacc, mybir, masks
from concourse.vector_clock import ScopedClock
from concourse.bass_utils import run_bass_kernel_spmd

f32 = mybir.dt.float32
f32r = mybir.dt.float32r
f16 = mybir.dt.float16
i16 = mybir.dt.int16
AF = mybir.ActivationFunctionType
ALU = mybir.AluOpType


class TC(tile.TileContext):
    """TileContext whose final drain splits sem waits into single-wait nops
    (walrus CoreV3 drain codegen rejects >2 wait commands per instruction)."""

    def _drain_and_barrier(self, tick_clock, wait_clock):
        nc = self.nc
        carrier = nc.sync.nop()
        wait_clock.add_sem_waits(
            carrier.ins, ScopedClock({None: tick_clock.global_clock})
        )
        si = carrier.ins.sync_info
        waits = list(si.on_wait) if si and si.on_wait else []
        if len(waits) > 2:
            si.on_wait = []
            for w in waits:
                nop = nc.sync.nop()
                nsi = nop.ins.sync_info
                if nsi is None:
                    nop.ins.sync_info = mybir.SyncInfo(on_wait=[w], on_update=[])
                else:
                    nsi.on_wait = [w]
        nc.sync.drain()
        nc.all_engine_barrier()
        assert self.sems is not None
        popped = nc._tile_sem_poison_stack.pop()
        assert popped is self._sem_poison
        nc.clear_and_free_semaphores(list(self.sems.allocated().values()))
        nc.all_engine_barrier()


# ---------------------------------------------------------------------------
# configuration
# ---------------------------------------------------------------------------

class Cfg:
    def __init__(self, N=3072, IN=256, DH=64, DV=32, H=4, ncores=8):
        P = 128
        self.N, self.IN, self.DH, self.DV, self.H, self.ncores = N, IN, DH, DV, H, ncores
        self.P = P
        assert N % (ncores * P) == 0
        self.RPC = N // ncores          # rows per core
        self.NT = self.RPC // P         # 128-row tiles per core
        self.HALF = N // 2              # scatter dst width
        assert self.HALF * 32 < 2**16, "local_scatter num_elems limit"
        # matmul chunking
        self.FCH = min(512, N)          # att matmul moving free chunk
        self.PIECE = min(1024, N)       # att psum piece width
        assert N % self.PIECE == 0 and self.PIECE % self.FCH == 0
        self.kchunks = []
        o = 0
        while o < IN:
            c = min(P, IN - o)
            self.kchunks.append((o, c))
            o += c
        self.kchunks.append((IN, 1))    # bias row
        self.INA = IN + 1
        self.VG = DV + 1                # v-group width (v columns + ones)
        self.NJ = N // P                # number of j chunks
        self.HPT = 2 if H >= 2 else 1   # heads per kt/qt tile
        self.NHP = H // self.HPT
        # two-level scatter layout — filled in by compute_layout(host data)
        self.layout = None

    def layout_key(self):
        lay = self.layout
        return (self.N, self.IN, self.DH, self.DV, self.H, self.ncores,
                lay["EC"], lay["SHARED_OFF"], lay["SHARED_END"],
                tuple(lay["copies"]))


# ---------------------------------------------------------------------------
# host preprocessing — two-level scatter schedule
# ---------------------------------------------------------------------------

CLS_Q, CLS_P, CLS_S, CLS_T, CLS_R, CLS_X = 0, 1, 2, 3, 4, 5
XTHR = 4  # kA or kB >= XTHR -> class X (rare; full-copy passes both sides)


def _classify(cfg, idx, edge):
    """Per (row, source) target counts and class for one branch."""
    N, HALF = cfg.N, cfg.HALF
    ii, jj = np.nonzero(edge)
    mm = idx[ii, jj].astype(np.int64)
    half = jj >= HALF
    jloc = np.where(half, jj - HALF, jj).astype(np.int16)
    key = ii.astype(np.int64) * N + mm
    uk, inv = np.unique(key, return_inverse=True)
    kA = np.bincount(inv, weights=(~half).astype(np.float64),
                     minlength=len(uk)).astype(np.int64)
    kB = np.bincount(inv, weights=half.astype(np.float64),
                     minlength=len(uk)).astype(np.int64)
    rows = (uk // N).astype(np.int64)
    cls = np.full(len(uk), -1, np.int64)
    cls[(kA >= 1) & (kB == 0)] = CLS_Q
    cls[(kA == 1) & (kB >= 1)] = CLS_P
    cls[(kA >= 2) & (kB == 1)] = CLS_S
    cls[(kA >= 2) & (kB >= 2)] = CLS_T
    cls[(kA == 0) & (kB >= 1)] = CLS_R
    cls[(kA >= XTHR) | (kB >= XTHR)] = CLS_X
    assert (cls >= 0).all()
    return dict(ii=ii, jj=jj, mm=mm, half=half, jloc=jloc, key=key, uk=uk,
                inv=inv, kA=kA, kB=kB, rows=rows, cls=cls)


def _rowmax(rows, mask, N):
    if not mask.any():
        return 0
    return int(np.bincount(rows[mask], minlength=N).max())


def _even(x):
    return int(x + (x & 1))


def compute_layout(cfg, cl_list, pad=1.0):
    """Derive the shared C-region layout from both branches' stats."""
    N = cfg.N

    def mx(f):
        return max(_rowmax(cl["rows"], f(cl), N) for cl in cl_list)

    LQ = _even(mx(lambda c: c["cls"] == CLS_Q) + int(8 * pad))
    LP = _even(mx(lambda c: c["cls"] == CLS_P) + int(8 * pad))
    LS = _even(mx(lambda c: c["cls"] == CLS_S) + int(4 * pad))
    LT = _even(mx(lambda c: c["cls"] == CLS_T) + int(2 * pad))
    LR = _even(mx(lambda c: c["cls"] == CLS_R) + int(8 * pad))
    LX = _even(mx(lambda c: c["cls"] == CLS_X) + int(2 * pad))

    def passlens(cname, kname, base_len, uniform=False):
        # prefix lengths for ranks r>=1: values with k >= r+1. Rank>=2
        # passes share one uniform length so their copies merge into a
        # single stride-0 broadcast tensor_copy (DVE instruction count).
        # uniform=True forces every pass to base_len (needed when the
        # region's sort key is not kname — prefix property would fail).
        out = []
        r = 1
        while True:
            n = mx(lambda c: (c["cls"] == cname) & (c[kname] >= r + 1))
            if n == 0:
                break
            out.append(_even(min(n + 4, base_len)))
            r += 1
        if uniform:
            return [base_len] * len(out)
        if len(out) > 1:
            u = out[1]
            out = [out[0]] + [u] * (len(out) - 1)
        return out

    QP = passlens(CLS_Q, "kA", LQ)
    SP = passlens(CLS_S, "kA", LS)
    TPA = passlens(CLS_T, "kA", LT, uniform=True)
    PB = passlens(CLS_P, "kB", LP)
    TB = passlens(CLS_T, "kB", LT, uniform=True)
    RP = passlens(CLS_R, "kB", LR)
    XA = passlens(CLS_X, "kA", LX, uniform=True)
    XB = passlens(CLS_X, "kB", LX, uniform=True)

    # absolute layout: [Q QP.. SP.. TPA.. XA.. | P S T X | PB.. TB.. XB.. R RP..]
    off = 0
    lay = {}
    lay["Q"] = off; off += LQ
    for nm, Ls in (("QP", QP), ("SP", SP), ("TPA", TPA), ("XA", XA)):
        lay[nm] = []
        for L in Ls:
            lay[nm].append(off); off += L
    lay["SHARED_OFF"] = off
    lay["P"] = off; off += LP
    lay["S"] = off; off += LS
    lay["T"] = off; off += LT
    lay["X"] = off; off += LX
    lay["SHARED_END"] = off
    for nm, Ls in (("PB", PB), ("TB", TB), ("XB", XB)):
        lay[nm] = []
        for L in Ls:
            lay[nm].append(off); off += L
    lay["R"] = off; off += LR
    lay["RP"] = []
    for L in RP:
        lay["RP"].append(off); off += L
    lay["EC"] = off
    assert off <= 2046, f"C region overflow: {off}"
    assert off * 32 < 2**16
    lay["lens"] = dict(Q=LQ, P=LP, S=LS, T=LT, R=LR, X=LX, QP=QP, SP=SP,
                       TPA=TPA, PB=PB, TB=TB, RP=RP, XA=XA, XB=XB)
    # DVE copy schedule: (dst_off, src_off, length, replicas). Consecutive
    # equal-length pass regions are emitted as one stride-0 broadcast copy.
    copies = []

    def emit(offs, lens_, src):
        i = 0
        while i < len(offs):
            k = 1
            while (i + k < len(offs) and lens_[i + k] == lens_[i]
                   and offs[i + k] == offs[i] + k * lens_[i]):
                k += 1
            copies.append((offs[i], src, lens_[i], k))
            i += k

    emit(lay["QP"], QP, lay["Q"])
    emit(lay["SP"], SP, lay["S"])
    emit(lay["TPA"], TPA, lay["T"])
    emit(lay["XA"], XA, lay["X"])
    emit(lay["PB"], PB, lay["P"])
    emit(lay["TB"], TB, lay["T"])
    emit(lay["XB"], XB, lay["X"])
    emit(lay["RP"], RP, lay["R"])
    lay["copies"] = tuple(copies)
    return lay


def build_schedule(cfg, cl):
    """Emit (cbuild, sA, sB) int16 index arrays for one branch."""
    N, HALF = cfg.N, cfg.HALF
    lay = cfg.layout
    lens = lay["lens"]
    uk, rows, cls = cl["uk"], cl["rows"], cl["cls"]
    kA, kB = cl["kA"], cl["kB"]

    # slot rank within (row, cls), sorted by count desc for prefix passes
    sortkey = np.where((cls == CLS_P) | (cls == CLS_R), kB, kA)
    order = np.lexsort((uk, -sortkey, cls, rows))
    ro, co = rows[order], cls[order]
    grp = np.r_[True, (ro[1:] != ro[:-1]) | (co[1:] != co[:-1])]
    gs = np.maximum.accumulate(np.where(grp, np.arange(len(ro)), 0))
    rank_sorted = np.arange(len(ro)) - gs
    slot_rel = np.empty(len(uk), np.int64)
    slot_rel[order] = rank_sorted

    base_off = np.array([lay["Q"], lay["P"], lay["S"], lay["T"], lay["R"],
                         lay["X"]])
    cap = np.array([lens["Q"], lens["P"], lens["S"], lens["T"], lens["R"],
                    lens["X"]])
    assert (slot_rel < cap[cls]).all(), "region capacity exceeded"
    slot = base_off[cls] + slot_rel

    cbuild = np.full((N, N), -1, np.int16)
    cbuild[rows, uk % N] = slot.astype(np.int16)

    # per-edge rank within (key, half)
    ii, jloc, half, key = cl["ii"], cl["jloc"], cl["half"], cl["key"]
    inv = cl["inv"]
    kh = key * 2 + half
    eorder = np.lexsort((jloc, kh))
    khs = kh[eorder]
    grp = np.r_[True, khs[1:] != khs[:-1]]
    gs = np.maximum.accumulate(np.where(grp, np.arange(len(khs)), 0))
    er_sorted = np.arange(len(khs)) - gs
    erank = np.empty(len(kh), np.int64)
    erank[eorder] = er_sorted

    e_cls = cls[inv]
    e_slot = slot[inv]
    e_rel = slot_rel[inv]
    e_row = ii.astype(np.int64)

    SHO, SHE, EC = lay["SHARED_OFF"], lay["SHARED_END"], lay["EC"]
    sA = np.full((N, SHE), -1, np.int16)
    sB = np.full((N, EC - SHO), -1, np.int16)

    # A-half deliveries
    a = ~half
    col = np.full(len(kh), -1, np.int64)
    r0 = a & (erank == 0)
    col[r0] = e_slot[r0]
    for roff_name, lens_name, cls_i in (("QP", "QP", CLS_Q),
                                        ("SP", "SP", CLS_S),
                                        ("TPA", "TPA", CLS_T),
                                        ("XA", "XA", CLS_X)):
        offs = lay[roff_name]
        plens = lay["lens"][lens_name]
        for r, o in enumerate(offs, start=1):
            sel = a & (erank == r) & (e_cls == cls_i)
            assert (e_rel[sel] < plens[r - 1]).all(), "A pass prefix overflow"
            col[sel] = o + e_rel[sel]
    bad = a & (col < 0)
    assert not bad.any(), "unmapped A delivery (rank beyond passes)"
    sA[e_row[a], col[a]] = jloc[a]

    # B-half deliveries (columns relative to SHARED_OFF)
    b = half
    colb = np.full(len(kh), -1, np.int64)
    r0 = b & (erank == 0) & (e_cls != CLS_Q)
    colb[r0] = e_slot[r0] - SHO
    for roff_name, cls_i in (("PB", CLS_P), ("TB", CLS_T), ("XB", CLS_X),
                             ("RP", CLS_R)):
        offs = lay[roff_name]
        plens = lay["lens"][roff_name]
        for r, o in enumerate(offs, start=1):
            sel = b & (erank == r) & (e_cls == cls_i)
            assert (e_rel[sel] < plens[r - 1]).all(), "B pass prefix overflow"
            colb[sel] = o - SHO + e_rel[sel]
    bad = b & (colb < 0)
    assert not bad.any(), "unmapped B delivery"
    sB[e_row[b], colb[b]] = jloc[b]
    return cbuild, sA, sB


def host_prep(cfg, inputs):
    """Returns per-core input maps (list of dicts); fills cfg.layout."""
    N, IN, DH, DV, H = cfg.N, cfg.IN, cfg.DH, cfg.DV, cfg.H
    x = np.asarray(inputs["x"], np.float32)
    fst = np.asarray(inputs["fst_graph"], np.float32)
    sec = np.asarray(inputs["sec_graph"], np.float32)
    n2c = np.asarray(inputs["n2c"]).astype(np.int32)
    c2n = np.asarray(inputs["c2n"]).astype(np.int32)

    scale = 1.0 / math.sqrt(DH)
    xTa = np.empty((IN + 1, N), np.float32)
    xTa[:IN] = x.T
    xTa[IN] = 1.0

    def aug(W, b, s=1.0):
        Wa = np.empty((IN + 1, W.shape[1]), np.float32)
        Wa[:IN] = np.asarray(W, np.float32) * s
        Wa[IN] = np.asarray(b, np.float32) * s
        return Wa

    wq1 = aug(inputs["Wq1"], inputs["bq1"], scale)
    wk1 = aug(inputs["Wk1"], inputs["bk1"])
    wq2 = aug(inputs["Wq2"], inputs["bq2"], scale)
    wk2 = aug(inputs["Wk2"], inputs["bk2"])
    # v' layout: per (branch b, head h) group of (DV+1) cols: [Wv_h | ones]
    VG = cfg.VG
    wva = np.zeros((IN + 1, 2 * H * VG), np.float32)
    for b, (Wv, bv) in enumerate(
        [(inputs["Wv1"], inputs["bv1"]), (inputs["Wv2"], inputs["bv2"])]
    ):
        Wv = np.asarray(Wv, np.float32)
        bv = np.asarray(bv, np.float32)
        for h in range(H):
            g = b * H + h
            wva[:IN, g * VG : g * VG + DV] = Wv[:, h * DV : (h + 1) * DV]
            wva[IN, g * VG : g * VG + DV] = bv[h * DV : (h + 1) * DV]
            wva[IN, g * VG + DV] = 1.0

    # branch 1 gathers att2 with c2n, edge1 = fst.T; branch 2 gathers att1
    # with n2c, edge2 = sec.T
    cl1 = _classify(cfg, c2n, fst.T != 0.0)
    cl2 = _classify(cfg, n2c, sec.T != 0.0)
    if cfg.layout is None:
        cfg.layout = compute_layout(cfg, [cl1, cl2])
    cb1, sa1, sb1 = build_schedule(cfg, cl1)
    cb2, sa2, sb2 = build_schedule(cfg, cl2)

    maps = []
    for c in range(cfg.ncores):
        r0, r1 = c * cfg.RPC, (c + 1) * cfg.RPC
        maps.append(dict(
            xta=xTa, wq1=wq1, wk1=wk1, wq2=wq2, wk2=wk2, wva=wva,
            cb1=np.ascontiguousarray(cb1[r0:r1]),
            sa1=np.ascontiguousarray(sa1[r0:r1]),
            sb1=np.ascontiguousarray(sb1[r0:r1]),
            cb2=np.ascontiguousarray(cb2[r0:r1]),
            sa2=np.ascontiguousarray(sa2[r0:r1]),
            sb2=np.ascontiguousarray(sb2[r0:r1]),
        ))
    return maps

# ---------------------------------------------------------------------------
# device kernel builder
# ---------------------------------------------------------------------------

def build_module(cfg, reps=1, skip=()):
    N, IN, DH, DV, H, P = cfg.N, cfg.IN, cfg.DH, cfg.DV, cfg.H, cfg.P
    NT, HALF = cfg.NT, cfg.HALF
    VG, NJ, INA, RPC = cfg.VG, cfg.NJ, cfg.INA, cfg.RPC
    lay = cfg.layout
    assert lay is not None, "call host_prep first (layout is data-derived)"
    EC, SHO, SHE = lay["EC"], lay["SHARED_OFF"], lay["SHARED_END"]
    LBS = EC - SHO
    nc = bacc.Bacc("TRN2", target_bir_lowering=False, debug=False,
                   num_devices=cfg.ncores)

    def dram_in(name, shape, dt):
        return nc.dram_tensor(name, list(shape), dt, kind="ExternalInput").ap()

    xta = dram_in("xta", (INA, N), f32)
    xtq = dram_in("xtq", (INA, RPC), f32)
    wq = [dram_in("wq1", (INA, H * DH), f32), dram_in("wq2", (INA, H * DH), f32)]
    wk = [dram_in("wk1", (INA, H * DH), f32), dram_in("wk2", (INA, H * DH), f32)]
    wva = dram_in("wva", (INA, 2 * H * VG), f32)
    cb_in = [dram_in("cb1", (RPC, N), i16), dram_in("cb2", (RPC, N), i16)]
    sa_in = [dram_in("sa1", (RPC, SHE), i16), dram_in("sa2", (RPC, SHE), i16)]
    sb_in = [dram_in("sb1", (RPC, LBS), i16), dram_in("sb2", (RPC, LBS), i16)]
    y = nc.dram_tensor("y", [RPC, 2 * H * DV], f32, kind="ExternalOutput").ap()

    HPD = cfg.HPT * DH
    with TC(nc) as tc:
        import contextlib
        with contextlib.ExitStack() as ctx:
            const_p = ctx.enter_context(tc.tile_pool(name="const", bufs=1))

            identf = const_p.tile([P, P], f32)
            masks.make_identity(nc, identf[:])
            identh = const_p.tile([P, P], f16)
            nc.vector.tensor_copy(identh[:], identf[:])
            expbias = const_p.tile([P, 1], f32)
            nc.gpsimd.memset(expbias[:], -1.5)

            # persistent projection outputs (fp16)
            kt = [[const_p.tile([HPD, N], f16, tag=f"kt{b}{hp}", name=f"kt{b}{hp}")
                   for hp in range(cfg.NHP)] for b in range(2)]
            qt = [[const_p.tile([HPD, RPC], f16, tag=f"qt{b}{hp}", name=f"qt{b}{hp}")
                   for hp in range(cfg.NHP)] for b in range(2)]
            VW = 2 * H * VG
            v_sb = const_p.tile([P, NJ * VW], f16)

            nkc = len(cfg.kchunks)
            # ---- projection phase (scoped pools, released afterwards) ----
            with tc.tile_pool(name="projsb", bufs=1) as proj_sb, \
                 tc.tile_pool(name="projps", bufs=2, space="PSUM") as proj_ps:
                xt, xq = [], []
                for o, csz in cfg.kchunks:
                    tf = proj_sb.tile([csz, N], f32, tag=f"xs{o}")
                    nc.sync.dma_start(tf[:], xta[o:o + csz, :])
                    tr = proj_sb.tile([csz, N], f32r, tag=f"xt{o}")
                    nc.vector.tensor_copy(tr[:], tf[:])
                    xt.append(tr)
                    tfq = proj_sb.tile([csz, RPC], f32, tag=f"xqs{o}")
                    nc.sync.dma_start(tfq[:], xtq[o:o + csz, :])
                    trq = proj_sb.tile([csz, RPC], f32r, tag=f"xq{o}")
                    nc.vector.tensor_copy(trq[:], tfq[:])
                    xq.append(trq)

                def load_w(ap, width, tag):
                    out = []
                    for o, csz in cfg.kchunks:
                        tf = proj_sb.tile([csz, width], f32, tag=f"{tag}s{o}")
                        nc.sync.dma_start(tf[:], ap[o:o + csz, :])
                        tr = proj_sb.tile([csz, width], f32r, tag=f"{tag}{o}")
                        nc.vector.tensor_copy(tr[:], tf[:])
                        out.append(tr)
                    return out

                wqt = [load_w(wq[b], H * DH, f"wq{b}") for b in range(2)]
                wkt = [load_w(wk[b], H * DH, f"wk{b}") for b in range(2)]
                wvt = load_w(wva, VW, "wv")

                for b in range(2):
                    for hp in range(cfg.NHP):
                        co = hp * HPD
                        for fc in range(0, N, cfg.FCH):
                            ps = proj_ps.tile([HPD, cfg.FCH], f32, tag="pk")
                            for kc in range(nkc):
                                nc.tensor.matmul(
                                    ps[:], wkt[b][kc][:, co:co + HPD],
                                    xt[kc][:, fc:fc + cfg.FCH],
                                    start=(kc == 0), stop=(kc == nkc - 1))
                            nc.scalar.copy(kt[b][hp][:, fc:fc + cfg.FCH], ps[:])
                        for fc in range(0, RPC, cfg.FCH):
                            fw = min(cfg.FCH, RPC - fc)
                            ps = proj_ps.tile([HPD, cfg.FCH], f32, tag="pq")
                            for kc in range(nkc):
                                nc.tensor.matmul(
                                    ps[:, 0:fw], wqt[b][kc][:, co:co + HPD],
                                    xq[kc][:, fc:fc + fw],
                                    start=(kc == 0), stop=(kc == nkc - 1))
                            nc.scalar.copy(qt[b][hp][:, fc:fc + fw], ps[:, 0:fw])
                for jc in range(NJ):
                    ps = proj_ps.tile([P, VW], f32, tag="pv")
                    for kc in range(nkc):
                        nc.tensor.matmul(
                            ps[:], xt[kc][:, jc * P:(jc + 1) * P], wvt[kc][:],
                            start=(kc == 0), stop=(kc == nkc - 1))
                    nc.scalar.copy(v_sb[:, jc * VW:(jc + 1) * VW], ps[:])

            # ---- main pools ----
            att_ps = ctx.enter_context(
                tc.tile_pool(name="att_ps", bufs=2, space="PSUM"))
            tp_ps = ctx.enter_context(
                tc.tile_pool(name="tp_ps", bufs=2, space="PSUM"))
            pv_ps = ctx.enter_context(
                tc.tile_pool(name="pv_ps", bufs=1, space="PSUM"))
            stream_p = ctx.enter_context(tc.tile_pool(name="stream", bufs=4))
            cb_p = ctx.enter_context(tc.tile_pool(name="cbidx", bufs=2))
            idx_p = ctx.enter_context(tc.tile_pool(name="idx", bufs=1))
            c_p = ctx.enter_context(tc.tile_pool(name="creg", bufs=3))
            g_p = ctx.enter_context(tc.tile_pool(name="gdst", bufs=2))
            p_p = ctx.enter_context(tc.tile_pool(name="p", bufs=2))
            st_p = ctx.enter_context(tc.tile_pool(name="st", bufs=3))
            out_p = ctx.enter_context(tc.tile_pool(name="out", bufs=2))
            sm_p = ctx.enter_context(tc.tile_pool(name="sm", bufs=4))

            for rep in range(reps):
              for t in range(NT):
                rt0 = t * P
                cbx = [cb_p.tile([P, N], i16, tag=f"cb{b}", name=f"cb{b}_{t}_{rep}")
                       for b in range(2)]
                sax = [idx_p.tile([P, SHE], i16, tag=f"sa{b}", name=f"sa{b}_{t}_{rep}")
                       for b in range(2)]
                sbx = [idx_p.tile([P, LBS], i16, tag=f"sb{b}", name=f"sb{b}_{t}_{rep}")
                       for b in range(2)]
                for b in range(2):
                    nc.sync.dma_start(cbx[b][:], cb_in[b][rt0:rt0 + P, :])
                    nc.sync.dma_start(sax[b][:], sa_in[b][rt0:rt0 + P, :])
                    nc.sync.dma_start(sbx[b][:], sb_in[b][rt0:rt0 + P, :])
                for h in range(H):
                    hp, ho = h // cfg.HPT, (h % cfg.HPT) * DH
                    streams = []
                    for b in range(2):
                        s = stream_p.tile([P, N], f16, tag=f"stream{b}")
                        if "att" in skip:
                            nc.gpsimd.memset(s[:, 0:2], 1.0)
                        else:
                            for po in range(0, N, cfg.PIECE):
                                ps = att_ps.tile([P, cfg.PIECE], f32, tag="attps")
                                for fo in range(0, cfg.PIECE, cfg.FCH):
                                    nc.tensor.matmul(
                                        ps[:, fo:fo + cfg.FCH],
                                        qt[b][hp][ho:ho + DH, rt0:rt0 + P],
                                        kt[b][hp][ho:ho + DH,
                                                  po + fo:po + fo + cfg.FCH],
                                        start=True, stop=True)
                                nc.scalar.activation(s[:, po:po + cfg.PIECE],
                                                     ps[:], AF.Exp, bias=expbias[:])
                        streams.append(s)
                    gdst = []
                    for b in range(2):
                        src = streams[1 - b]
                        creg = c_p.tile([P, EC], f16, tag=f"c{b}",
                                        name=f"creg{b}_{t}_{h}_{rep}")
                        gd = g_p.tile([P, N], f16, tag=f"gd{b}",
                                      name=f"gd{b}_{t}_{h}_{rep}")
                        if "scatter" not in skip:
                            nreps = 2 if "dblbuild" in skip else 1
                            for _ in range(nreps):
                                nc.gpsimd.local_scatter(
                                    creg[:], src[:, 0:N], cbx[b][:],
                                    channels=P, num_elems=EC, num_idxs=N)
                            for cdst, csrc, clen, krep in lay["copies"]:
                                if krep == 1:
                                    nc.vector.tensor_copy(
                                        creg[:, cdst:cdst + clen],
                                        creg[:, csrc:csrc + clen])
                                else:
                                    dst3 = creg[:, cdst:cdst + krep * clen] \
                                        .rearrange("p (k l) -> p k l", k=krep)
                                    src3 = creg[:, csrc:csrc + clen] \
                                        .unsqueeze(1) \
                                        .broadcast_to([P, krep, clen])
                                    nc.vector.tensor_copy(dst3, src3)
                            nreps = 2 if "dblscan" in skip else 1
                            for _ in range(nreps):
                                nc.gpsimd.local_scatter(
                                    gd[:, 0:HALF], creg[:, 0:SHE], sax[b][:],
                                    channels=P, num_elems=HALF, num_idxs=SHE)
                                nc.gpsimd.local_scatter(
                                    gd[:, HALF:N], creg[:, SHO:EC], sbx[b][:],
                                    channels=P, num_elems=HALF, num_idxs=LBS)
                        else:
                            nc.gpsimd.memset(gd[:, 0:2], 1.0)
                        gdst.append(gd)
                    for b in range(2):
                        pv = pv_ps.tile([VG, P], f32, tag="pv",
                                        name=f"pv{b}_{t}_{h}_{rep}")
                        if "pv" in skip:
                            nc.tensor.matmul(pv[:], v_sb[:, 0:VG], identh[:],
                                             start=True, stop=True)
                        else:
                            sf = p_p.tile([P, N], f16, tag="p",
                                          name=f"sf{b}_{t}_{h}_{rep}")
                            nc.vector.tensor_mul(sf[:], streams[b][:],
                                                 gdst[b][:])
                            g_v = b * H + h
                            GRP = 8
                            for jg in range(0, NJ, GRP):
                                gn = min(GRP, NJ - jg)
                                tp = tp_ps.tile([P, GRP * P], f16, tag="tp",
                                                name=f"tp{b}_{t}_{h}_{jg}_{rep}")
                                for q in range(gn):
                                    nc.tensor.transpose(
                                        tp[:, q * P:(q + 1) * P],
                                        sf[:, (jg + q) * P:(jg + q + 1) * P],
                                        identh[:])
                                stt = st_p.tile([P, GRP * P], f16, tag="stt",
                                                name=f"stt{b}_{t}_{h}_{jg}_{rep}")
                                if (jg // GRP) % 3 == 2:
                                    nc.scalar.copy(stt[:, 0:gn * P],
                                                   tp[:, 0:gn * P])
                                else:
                                    nc.vector.tensor_copy(stt[:, 0:gn * P],
                                                          tp[:, 0:gn * P])
                                for q in range(gn):
                                    jc = jg + q
                                    nc.tensor.matmul(
                                        pv[:], v_sb[:, jc * VW + g_v * VG:
                                                    jc * VW + (g_v + 1) * VG],
                                        stt[:, q * P:(q + 1) * P],
                                        start=(jc == 0), stop=(jc == NJ - 1))
                        pvs = out_p.tile([VG, P], f32, tag="pvs")
                        nc.vector.tensor_copy(pvs[:], pv[:])
                        ot = pv_ps.tile([P, VG], f32, tag="otp")
                        nc.tensor.transpose(ot[:], pvs[:], identf[0:VG, 0:VG])
                        rec = sm_p.tile([P, 1], f32, tag="rec")
                        nc.vector.reciprocal(rec[:], ot[:, DV:DV + 1])
                        res = out_p.tile([P, DV], f32, tag="res")
                        nc.vector.tensor_mul(res[:], ot[:, 0:DV],
                                             rec[:].broadcast_to([P, DV]))
                        nc.sync.dma_start(
                            y[rt0:rt0 + P, (b * H + h) * DV:
                              (b * H + h + 1) * DV], res[:])
    nc.compile()
    return nc


# ---------------------------------------------------------------------------
# entry point
# ---------------------------------------------------------------------------

_CACHE = {}


def _get_module(cfg):
    key = cfg.layout_key()
    if key not in _CACHE:
        _CACHE[key] = build_module(cfg)
    return _CACHE[key]


def kernel(**inputs):
    """Full-input entry point: shards across 8 NeuronCores internally and
    returns the full (N, 2*H*DV) float32 output."""
    cfg = Cfg(N=3072, IN=256, DH=64, DV=32, H=4, ncores=8)
    maps = host_prep(cfg, inputs)
    nc = _get_module(cfg)
    for c, m in enumerate(maps):
        r0 = c * cfg.RPC
        m["xtq"] = np.ascontiguousarray(m["xta"][:, r0:r0 + cfg.RPC])
    res = run_bass_kernel_spmd(nc, maps, list(range(cfg.ncores)), trace=False)
    out = np.concatenate(
        [res.results[c]["y"] for c in range(cfg.ncores)], axis=0)
    return out.astype(np.float32)
